# revision 11
# baseline (speedup 1.0000x reference)
"""Trainium2 Bass kernel for BaselineProtonet (retrieval_knn).

logits[q, c] = -||query_q - proto_c||_2
  proto_c = mean of 64 support embeddings of class c
  embeddings_stacked: [64 classes * (64 support + 64 query), 1024] f32

Sharding (8 cores): 2D-balanced grid, 4 query-groups x 2 class-halves.
Core (a, b) owns query rows 1024a..1024(a+1) and classes 32b..32b+32, so
it reads 1MB of queries + 2MB of support (both fp8 on the wire) instead
of the 4.5MB a pure query shard would need -- the input DMA is the
dominant cost and this is the byte-optimal integer grid. No cross-core
collective (a ncfw collective costs ~65us of control latency in this
runtime, measured).

Host-side shard prep (layout/encoding only, no arithmetic): support is
stored SLOT-major (shard row r holds support vector (class r%32, slot
r//32)) so the one-hot stationary is the same for every proto matmul
(class of row == partition%32 -- one LDWEIGHTS for the whole stream),
pre-swizzled into two d-halves so the evac/transpose/W chain of half 0
runs during half 1's DMA; queries are feature-major fp8 (the kernel
uses the rounded values consistently in the Gram and ||q||^2 terms, so
fp8 queries shift each distance, not decorrelate the terms).

Per core:
  protos   : fp8 DoubleRow one-hot matmuls (256 support rows each, one
             shared stationary) accumulate class sums, one PSUM tile
             per d-half so the evac of half 0 doesn't wait on half 1
  P^T      : per d-half ACT evac (1/64 -> bf16), 4 PE transposes, ACT
             scale -2 -> W fp8 [128, 8, 32]
  Gram     : 8 fp8 DoubleRow matmuls lhsT=W pair, rhs=Q^T pair; the
             d-half-0 pairs OPEN the s_ps group early
  ||q||^2  : fp8 squares split DVE(6)/ACT(2); 8 all-ones fp8 DoubleRow
             matmuls broadcast-sum them into s_ps (no DVE add tree; DVE
             squares run ~1.2us per [128,1024] chunk, so they get the
             whole support phase to finish)
  ||p||^2  : ACT square-accumulate on the bf16 prototypes -> [32,1] f32
  logits   : -sqrt(dist^2) via ACT sqrt(+||p||^2 bias) and DVE negate,
             four quarters, each stored by its own DMA as it finishes;
             output [32, 1024] class-major, host transposes into place.
Support DMAs (4-chunk granularity, so matmuls track the stream) ride
the sync ring; one-hot + queries ride the scalar ring. PE pre-warmed
with dummy matmuls (HAM clock gate); ACT tables preloaded early.
"""

import numpy as np

C = 64          # classes
S = 64          # support per class (== queries per class)
D = 1024        # embedding dim
NCORES = 8
QA = 4          # query groups
CB = 2          # class halves
CL = C // CB    # 32 classes per core
NQ = (C * S) // QA          # 1024 query rows per core
DCH = D // 128              # 8 d-chunks
SCH = (CL * S) // 128       # 16 support row chunks per core
JP = SCH // 2               # 8 DoubleRow chunk pairs

_CACHE = {}


def _emit(nc, tc, sup, qt, out):
    """Emit the per-core tile program.

    sup:   [128, 2*SCH*512] fp8 DRAM  (support, slot-major rows, d-half-
                                       major: half h, chunk j, d-slice;
                                       row p of chunk j = shard row
                                       j*128+p = class (j*128+p)%32)
    qt:    [128, DCH*NQ] fp8 DRAM     (queries, swizzled feature-major)
    out:   [CL, NQ] f32 DRAM          (negated distances, class-major)
    """
    from concourse import masks, mybir

    f32 = mybir.dt.float32
    bf16 = mybir.dt.bfloat16
    fp8 = mybir.dt.float8e4
    AF = mybir.ActivationFunctionType
    DR = mybir.MatmulPerfMode.DoubleRow

    with (
        tc.tile_pool(name="sb", bufs=1) as sb,
        tc.tile_pool(name="ps", bufs=1, space="PSUM") as ps,
    ):
        # warm the PE clock first-thing (HAM gate needs ~3.5us of busy
        # before the real matmuls; deps are a single DVE memset)
        wm_in = sb.tile([128, 512], bf16)
        nc.vector.memset(wm_in[:], 0.0)
        wm_ps = ps.tile([128, 512], f32)
        for _ in range(11):
            nc.tensor.matmul(
                wm_ps[:], wm_in[:, 0:128], wm_in[:], start=True, stop=True
            )

        # ---------------- input DMAs, ALL on the sync ring in 512KB
        # starts (one ring + few big starts avoids the cross-ring
        # DMA-semaphore-lane false waits seen in profiling; this is the
        # pattern that sustains ~390 GB/s). First query half first (it
        # feeds the slow square chain), then support d-half 0, the rest
        # of the queries, support d-half 1.
        q8 = sb.tile([128, DCH, NQ], fp8)
        s8 = sb.tile([128, 2, SCH, 512], fp8)

        def q_dma(g):
            nc.sync.dma_start(
                q8[:, 4 * g : 4 * (g + 1)],
                qt[:, 4 * g * NQ : 4 * (g + 1) * NQ].rearrange(
                    "p (k q) -> p k q", k=4
                ),
            )

        def s_dma(h, lo):
            nc.sync.dma_start(
                s8[:, h, lo : lo + 8],
                sup[
                    :, (h * SCH + lo) * 512 : (h * SCH + lo + 8) * 512
                ].rearrange("p (c d) -> p c d", c=8),
            )

        q_dma(0)
        s_dma(0, 0)
        s_dma(0, 8)
        q_dma(1)
        s_dma(1, 0)
        s_dma(1, 8)

        # ---------------- constants -------------------------------------
        ident = sb.tile([128, 128], bf16)
        masks.make_identity(nc, ident[:])
        ones = sb.tile([128, 2, CL], fp8)
        nc.gpsimd.memset(ones[:], 1.0)
        # one-hot built on device: oh[p, o, c] = 1 iff c == p % 32,
        # i.e. p - c - 32k == 0 for some k (four diagonal stripe fills)
        oh = sb.tile([128, 2, CL], fp8)
        nc.gpsimd.memset(oh[:], 0.0)
        for k4 in range(4):
            nc.gpsimd.affine_select(
                out=oh[:],
                in_=oh[:],
                compare_op=mybir.AluOpType.not_equal,
                fill=1.0,
                base=-CL * k4,
                pattern=[[0, 2], [-1, CL]],
                channel_multiplier=1,
            )

        # preload the ACT tables off the critical path (Copy for the
        # evacs, Square for ||p||^2/||q||^2, Sqrt for the distances)
        warm_t = sb.tile([1, 1], f32)
        warm_d = sb.tile([1, 1], bf16)
        warm_a = sb.tile([1, 1], f32)
        nc.gpsimd.memset(warm_t[:], 1.0)
        nc.scalar.mul(warm_d[:], warm_t[:], 1.0)
        nc.scalar.activation(warm_d[:], warm_t[:], AF.Square, accum_out=warm_a[:])
        nc.scalar.activation(warm_t[:], warm_t[:], AF.Sqrt)

        # ---------------- prototypes (fp8 DoubleRow, shared stationary,
        # one PSUM tile per d-half) --------------------------------------
        s8v = s8[:].rearrange("p h (jp o) d -> p h jp o d", o=2)
        p_half = [ps.tile([CL, 512], f32, name=f"p_ps{h}") for h in range(2)]
        for jp in range(JP):
            nc.tensor.matmul(
                p_half[0][:],
                oh[:],
                s8v[:, 0, jp],
                start=(jp == 0),
                stop=(jp == JP - 1),
                perf_mode=DR,
            )

        # evac d-half 0, transposes, W fp8 = -2 * P^T
        psbA = sb.tile([CL, 512], bf16)
        psbB = sb.tile([CL, 512], bf16)
        nc.scalar.mul(psbA[:], p_half[0][:], 1.0 / S)
        pt_ps = ps.tile([128, DCH, CL], bf16)
        W = sb.tile([128, DCH, CL], fp8)
        for k in range(4):
            nc.tensor.transpose(
                pt_ps[:, k], psbA[:, 128 * k : 128 * (k + 1)], ident[0:CL, 0:CL]
            )
        nc.scalar.mul(W[:, 0:4], pt_ps[:, 0:4], -2.0)

        # prototypes d-half 1
        for jp in range(JP):
            nc.tensor.matmul(
                p_half[1][:],
                oh[:],
                s8v[:, 1, jp],
                start=(jp == 0),
                stop=(jp == JP - 1),
                perf_mode=DR,
            )

        # Gram d-half-0 pairs OPEN the s_ps group
        s_ps = ps.tile([CL, NQ], f32)
        q8v = q8[:].rearrange("p (kp o) q -> p kp o q", o=2)
        for n in range(2):
            for kp in range(2):
                nc.tensor.matmul(
                    s_ps[:, 512 * n : 512 * (n + 1)],
                    W[:, 2 * kp : 2 * kp + 2],
                    q8v[:, kp, :, 512 * n : 512 * (n + 1)],
                    start=(kp == 0),
                    stop=False,
                    perf_mode=DR,
                )

        # ---------------- ||q||^2 squares: DVE chunks 0-3,6,7; ACT 4,5;
        # fp8 out so the ones-matmuls below can use DoubleRow ------------
        qsq = sb.tile([128, DCH, NQ], fp8)
        for k in (0, 1, 2, 3):
            nc.vector.tensor_mul(qsq[:, k], q8[:, k], q8[:, k])
        for k in (4, 5):
            nc.scalar.activation(qsq[:, k], q8[:, k], AF.Square)
        for k in (6, 7):
            nc.vector.tensor_mul(qsq[:, k], q8[:, k], q8[:, k])

        # evac d-half 1, transposes, W
        nc.scalar.mul(psbB[:], p_half[1][:], 1.0 / S)
        for k in range(4, 8):
            nc.tensor.transpose(
                pt_ps[:, k],
                psbB[:, 128 * (k - 4) : 128 * (k - 3)],
                ident[0:CL, 0:CL],
            )
        nc.scalar.mul(W[:, 4:8], pt_ps[:, 4:8], -2.0)

        # ||p||^2 in f32 via ACT square-accumulate on the bf16 protos
        pn_dump = sb.tile([CL, D], bf16)
        pnA = sb.tile([CL, 1], f32)
        pnB = sb.tile([CL, 1], f32)
        pn_col = sb.tile([CL, 1], f32)
        nc.scalar.activation(pn_dump[:, 0:512], psbA[:], AF.Square, accum_out=pnA[:])
        nc.scalar.activation(
            pn_dump[:, 512:1024], psbB[:], AF.Square, accum_out=pnB[:]
        )
        nc.vector.tensor_add(pn_col[:], pnA[:], pnB[:])

        # ||q||^2 ones-matmuls (fp8 DoubleRow over qsq chunk pairs; ACT
        # chunks 4,5 first, the last DVE chunks last) and the Gram
        # d-half-1 pairs, n-region-major so n=0 closes as early as
        # possible for the sqrt/store pipeline
        qsqv = qsq[:].rearrange("p (m o) q -> p m o q", o=2)
        for n in range(2):
            for m in (2, 0, 1, 3):
                nc.tensor.matmul(
                    s_ps[:, 512 * n : 512 * (n + 1)],
                    ones[:],
                    qsqv[:, m, :, 512 * n : 512 * (n + 1)],
                    start=False,
                    stop=False,
                    perf_mode=DR,
                )
            for kp in (2, 3):
                nc.tensor.matmul(
                    s_ps[:, 512 * n : 512 * (n + 1)],
                    W[:, 2 * kp : 2 * kp + 2],
                    q8v[:, kp, :, 512 * n : 512 * (n + 1)],
                    start=False,
                    stop=(kp == 3),
                    perf_mode=DR,
                )

        # ------- sqrt(+||p||^2), negate, store (quarters pipelined) -----
        lt = sb.tile([CL, NQ], f32)
        for qi in range(4):
            s = slice(256 * qi, 256 * (qi + 1))
            nc.scalar.activation(
                lt[:, s], s_ps[:, s], AF.Sqrt, bias=pn_col[:, 0:1]
            )
            nc.vector.tensor_scalar_mul(lt[:, s], lt[:, s], -1.0)
            nc.sync.dma_start(out[:, s], lt[:, s])


def _build():
    if "nc" in _CACHE:
        return _CACHE["nc"]
    from concourse import bacc, mybir, tile

    f32 = mybir.dt.float32
    fp8 = mybir.dt.float8e4
    nc = bacc.Bacc(
        "TRN2",
        target_bir_lowering=False,
        debug=False,
        enable_asserts=False,
        num_devices=NCORES,
    )
    sup = nc.dram_tensor("sup", [128, 2 * SCH * 512], fp8, kind="ExternalInput").ap()
    qt = nc.dram_tensor("qt", [128, DCH * NQ], fp8, kind="ExternalInput").ap()
    out = nc.dram_tensor("out", [CL, NQ], f32, kind="ExternalOutput").ap()
    with tile.TileContext(nc) as tc:
        _emit(nc, tc, sup, qt, out)
    nc.compile()
    _CACHE["nc"] = nc
    return nc


def _shard(embeddings):
    import ml_dtypes

    emb = np.asarray(embeddings, dtype=np.float32).reshape(C, 2 * S, D)
    # support halves: classes 32b..32b+32, SLOT-major shard rows
    # (r = s*32 + c_local), swizzled [128, 2, SCH, 512] d-half-major
    # (row p of chunk j = shard row j*128+p), fp8
    sups = []
    for b in range(CB):
        shard = (
            emb[CL * b : CL * (b + 1), :S, :]
            .transpose(1, 0, 2)
            .reshape(CL * S, D)
        )
        sw = shard.reshape(SCH, 128, 2, 512).transpose(1, 2, 0, 3)
        sups.append(
            np.ascontiguousarray(
                sw.astype(ml_dtypes.float8_e4m3).reshape(128, 2 * SCH * 512)
            )
        )
    # query groups: rows 1024a..1024(a+1) of the query set, feature-major
    qry = emb[:, S:, :].reshape(C * S, D)
    qts = []
    for a in range(QA):
        q = qry[NQ * a : NQ * (a + 1)]
        qt_i = q.T.reshape(DCH, 128, NQ).transpose(1, 0, 2)
        qts.append(
            np.ascontiguousarray(
                qt_i.astype(ml_dtypes.float8_e4m3).reshape(128, DCH * NQ)
            )
        )
    in_maps = []
    for i in range(NCORES):
        a, b = i // CB, i % CB
        in_maps.append({"sup": sups[b], "qt": qts[a]})
    return in_maps


def _assemble(outs):
    """outs: per-core [CL, NQ] blocks -> full [C*S, C] logits."""
    logits = np.empty((C * S, C), dtype=np.float32)
    for i in range(NCORES):
        a, b = i // CB, i % CB
        logits[NQ * a : NQ * (a + 1), CL * b : CL * (b + 1)] = outs[i].T
    return logits


def kernel(embeddings_stacked, n_classes, n_support, **_unused):
    assert int(n_classes) == C and int(n_support) == S
    emb = np.asarray(embeddings_stacked)
    assert emb.shape == (C * 2 * S, D), emb.shape

    from concourse import bass_utils

    nc = _build()
    in_maps = _shard(emb)
    try:
        res = bass_utils.run_bass_kernel_spmd(
            nc, in_maps, core_ids=list(range(NCORES))
        )
    except Exception:
        # transient device/runtime hiccups have been observed; retry once
        res = bass_utils.run_bass_kernel_spmd(
            nc, in_maps, core_ids=list(range(NCORES))
        )
    return _assemble([res.results[i]["out"] for i in range(NCORES)])


if __name__ == "__main__":
    rng = np.random.default_rng(0)
    emb = rng.standard_normal((C * 2 * S, D), dtype=np.float32)
    got = kernel(emb, C, S)
    print("kernel output", got.shape, got.dtype)


# revision 13
# speedup vs baseline: 1.0853x; 1.0853x over previous
"""Trainium2 Bass kernel for BaselineProtonet (retrieval_knn).

logits[q, c] = -||query_q - proto_c||_2
  proto_c = mean of 64 support embeddings of class c
  embeddings_stacked: [64 classes * (64 support + 64 query), 1024] f32

Sharding (8 cores): 2D-balanced grid, 4 query-groups x 2 class-halves.
Core (a, b) owns query rows 1024a..1024(a+1) and classes 32b..32b+32, so
it reads 1MB of queries + 2MB of support (both fp8 on the wire) instead
of the 4.5MB a pure query shard would need -- the input DMA is the
dominant cost and this is the byte-optimal integer grid. No cross-core
collective (a ncfw collective costs ~65us of control latency in this
runtime, measured).

Host-side shard prep (layout/encoding only, no arithmetic): support is
stored SLOT-major (shard row r holds support vector (class r%32, slot
r//32)) so the one-hot stationary is the same for every proto matmul
(class of row == partition%32 -- one LDWEIGHTS for the whole stream),
pre-swizzled into two d-halves so the evac/transpose/W chain of half 0
runs during half 1's DMA; queries are feature-major fp8 (the kernel
uses the rounded values consistently in the Gram and ||q||^2 terms, so
fp8 queries shift each distance, not decorrelate the terms).

Per core:
  protos   : fp8 DoubleRow one-hot matmuls (256 support rows each, one
             shared stationary) accumulate class sums, one PSUM tile
             per d-half so the evac of half 0 doesn't wait on half 1
  P^T      : per d-half ACT evac (1/64 -> bf16), 4 PE transposes, ACT
             scale -2 -> W fp8 [128, 8, 32]
  Gram     : 8 fp8 DoubleRow matmuls lhsT=W pair, rhs=Q^T pair; the
             d-half-0 pairs OPEN the s_ps group early
  ||q||^2  : fp8 squares split DVE(6)/ACT(2); 8 all-ones fp8 DoubleRow
             matmuls broadcast-sum them into s_ps (no DVE add tree; DVE
             squares run ~1.2us per [128,1024] chunk, so they get the
             whole support phase to finish)
  ||p||^2  : ACT square-accumulate on the bf16 prototypes -> [32,1] f32
  logits   : -sqrt(dist^2) via ACT sqrt(+||p||^2 bias) and DVE negate,
             four quarters, each stored by its own DMA as it finishes;
             output [32, 1024] class-major, host transposes into place.
Support DMAs (4-chunk granularity, so matmuls track the stream) ride
the sync ring; one-hot + queries ride the scalar ring. PE pre-warmed
with dummy matmuls (HAM clock gate); ACT tables preloaded early.
"""

import numpy as np

C = 64          # classes
S = 64          # support per class (== queries per class)
D = 1024        # embedding dim
NCORES = 8
QA = 4          # query groups
CB = 2          # class halves
CL = C // CB    # 32 classes per core
NQ = (C * S) // QA          # 1024 query rows per core
DCH = D // 128              # 8 d-chunks
SCH = (CL * S) // 128       # 16 support row chunks per core
JP = SCH // 2               # 8 DoubleRow chunk pairs

_CACHE = {}


def _emit(nc, tc, sup, qt, out):
    """Emit the per-core tile program.

    sup:   [128, 2*SCH*512] fp8 DRAM  (support, slot-major rows, d-half-
                                       major: half h, chunk j, d-slice;
                                       row p of chunk j = shard row
                                       j*128+p = class (j*128+p)%32)
    qt:    [128, DCH*NQ] fp8 DRAM     (queries, swizzled feature-major)
    out:   [CL, NQ] f32 DRAM          (negated distances, class-major)
    """
    from concourse import masks, mybir

    f32 = mybir.dt.float32
    bf16 = mybir.dt.bfloat16
    fp8 = mybir.dt.float8e4
    AF = mybir.ActivationFunctionType
    DR = mybir.MatmulPerfMode.DoubleRow

    with (
        tc.tile_pool(name="sb", bufs=1) as sb,
        tc.tile_pool(name="ps", bufs=1, space="PSUM") as ps,
    ):
        # warm the PE clock first-thing (HAM gate needs ~3.5us of busy
        # before the real matmuls; deps are a single DVE memset)
        wm_in = sb.tile([128, 512], bf16)
        nc.vector.memset(wm_in[:], 0.0)
        wm_ps = ps.tile([128, 512], f32)
        for _ in range(20):
            nc.tensor.matmul(
                wm_ps[:, 0:128], wm_in[:, 0:128], wm_in[:, 0:128],
                start=True, stop=True,
            )

        # ---------------- input DMAs, ALL on the sync ring in 512KB
        # starts (one ring + few big starts avoids the cross-ring
        # DMA-semaphore-lane false waits seen in profiling; this is the
        # pattern that sustains ~390 GB/s). First query half first (it
        # feeds the slow square chain), then support d-half 0, the rest
        # of the queries, support d-half 1.
        q8 = sb.tile([128, DCH, NQ], fp8)
        s8 = sb.tile([128, 2, SCH, 512], fp8)

        def q_dma(lo, hi):
            nc.sync.dma_start(
                q8[:, lo:hi],
                qt[:, lo * NQ : hi * NQ].rearrange(
                    "p (k q) -> p k q", k=hi - lo
                ),
            )

        def s_dma(h, lo):
            nc.sync.dma_start(
                s8[:, h, lo : lo + 8],
                sup[
                    :, (h * SCH + lo) * 512 : (h * SCH + lo + 8) * 512
                ].rearrange("p (c d) -> p c d", c=8),
            )

        q_dma(0, 6)
        s_dma(0, 0)
        s_dma(0, 8)
        q_dma(6, 8)
        s_dma(1, 0)
        s_dma(1, 8)

        # ---------------- constants -------------------------------------
        ident = sb.tile([128, 128], bf16)
        masks.make_identity(nc, ident[:])
        ones = sb.tile([128, 2, CL], fp8)
        nc.gpsimd.memset(ones[:], 1.0)
        # one-hot built on device: oh[p, o, c] = 1 iff c == p % 32,
        # i.e. p - c - 32k == 0 for some k (four diagonal stripe fills)
        oh = sb.tile([128, 2, CL], fp8)
        nc.gpsimd.memset(oh[:], 0.0)
        for k4 in range(4):
            nc.gpsimd.affine_select(
                out=oh[:],
                in_=oh[:],
                compare_op=mybir.AluOpType.not_equal,
                fill=1.0,
                base=-CL * k4,
                pattern=[[0, 2], [-1, CL]],
                channel_multiplier=1,
            )

        # preload the ACT tables off the critical path (Copy for the
        # evacs, Square for ||p||^2/||q||^2, Sqrt for the distances)
        warm_t = sb.tile([1, 1], f32)
        warm_d = sb.tile([1, 1], bf16)
        warm_a = sb.tile([1, 1], f32)
        nc.gpsimd.memset(warm_t[:], 1.0)
        nc.scalar.mul(warm_d[:], warm_t[:], 1.0)
        nc.scalar.activation(warm_d[:], warm_t[:], AF.Square, accum_out=warm_a[:])
        nc.scalar.activation(warm_t[:], warm_t[:], AF.Sqrt)

        # ---------------- prototypes (fp8 DoubleRow, shared stationary,
        # one PSUM tile per d-half) --------------------------------------
        s8v = s8[:].rearrange("p h (jp o) d -> p h jp o d", o=2)
        p_half = [ps.tile([CL, 512], f32, name=f"p_ps{h}") for h in range(2)]
        for jp in range(JP):
            nc.tensor.matmul(
                p_half[0][:],
                oh[:],
                s8v[:, 0, jp],
                start=(jp == 0),
                stop=(jp == JP - 1),
                perf_mode=DR,
            )

        # ||q||^2 squares (fp8 out, feeding the DoubleRow ones-matmuls):
        # ACT takes chunks 4,5 (early, before the evac work lands) and 6;
        # DVE takes 0-3 and 7. Emitted around the evac chain so the ACT
        # queue order is [sq4, sq5, evacA, WscA, sq6, pnA, evacB, ...].
        qsq = sb.tile([128, DCH, NQ], fp8)
        for k in (4, 5):
            nc.scalar.activation(qsq[:, k], q8[:, k], AF.Square)
        for k in (0, 1, 2, 3):
            nc.vector.tensor_mul(qsq[:, k], q8[:, k], q8[:, k])

        # evac d-half 0, transposes, W fp8 = -2 * P^T
        psbA = sb.tile([CL, 512], bf16)
        psbB = sb.tile([CL, 512], bf16)
        nc.scalar.mul(psbA[:], p_half[0][:], 1.0 / S)
        pt_ps = ps.tile([128, DCH, CL], bf16)
        W = sb.tile([128, DCH, CL], fp8)
        for k in range(4):
            nc.tensor.transpose(
                pt_ps[:, k], psbA[:, 128 * k : 128 * (k + 1)], ident[0:CL, 0:CL]
            )
        nc.scalar.mul(W[:, 0:4], pt_ps[:, 0:4], -2.0)
        nc.scalar.activation(qsq[:, 6], q8[:, 6], AF.Square)
        nc.vector.tensor_mul(qsq[:, 7], q8[:, 7], q8[:, 7])

        # ||p||^2 half 0 (ACT square-accumulate on the bf16 protos)
        pn_dump = sb.tile([CL, D], bf16)
        pnA = sb.tile([CL, 1], f32)
        pnB = sb.tile([CL, 1], f32)
        pn_col = sb.tile([CL, 1], f32)
        nc.scalar.activation(pn_dump[:, 0:512], psbA[:], AF.Square, accum_out=pnA[:])

        # prototypes d-half 1
        for jp in range(JP):
            nc.tensor.matmul(
                p_half[1][:],
                oh[:],
                s8v[:, 1, jp],
                start=(jp == 0),
                stop=(jp == JP - 1),
                perf_mode=DR,
            )

        # Gram d-half-0 pairs OPEN the s_ps group
        s_ps = ps.tile([CL, NQ], f32)
        q8v = q8[:].rearrange("p (kp o) q -> p kp o q", o=2)
        for n in range(2):
            for kp in range(2):
                nc.tensor.matmul(
                    s_ps[:, 512 * n : 512 * (n + 1)],
                    W[:, 2 * kp : 2 * kp + 2],
                    q8v[:, kp, :, 512 * n : 512 * (n + 1)],
                    start=(kp == 0),
                    stop=False,
                    perf_mode=DR,
                )

        # ||q||^2 ones-matmuls for n=0 (fp8 DoubleRow over chunk pairs)
        # run in the PE gap right after the d-half-1 protos
        qsqv = qsq[:].rearrange("p (m o) q -> p m o q", o=2)

        def qsq_mm(n):
            for m in range(4):
                nc.tensor.matmul(
                    s_ps[:, 512 * n : 512 * (n + 1)],
                    ones[:],
                    qsqv[:, m, :, 512 * n : 512 * (n + 1)],
                    start=False,
                    stop=False,
                    perf_mode=DR,
                )

        qsq_mm(0)

        # evac d-half 1, transposes, ||p||^2 half 1, W
        nc.scalar.mul(psbB[:], p_half[1][:], 1.0 / S)
        for k in range(4, 8):
            nc.tensor.transpose(
                pt_ps[:, k],
                psbB[:, 128 * (k - 4) : 128 * (k - 3)],
                ident[0:CL, 0:CL],
            )
        nc.scalar.activation(
            pn_dump[:, 512:1024], psbB[:], AF.Square, accum_out=pnB[:]
        )
        nc.vector.tensor_add(pn_col[:], pnA[:], pnB[:])
        nc.scalar.mul(W[:, 4:8], pt_ps[:, 4:8], -2.0)

        # Gram d-half-1 pairs: close n=0 first (early sqrt), then n=1
        def gram23(n, stop):
            for kp in (2, 3):
                nc.tensor.matmul(
                    s_ps[:, 512 * n : 512 * (n + 1)],
                    W[:, 2 * kp : 2 * kp + 2],
                    q8v[:, kp, :, 512 * n : 512 * (n + 1)],
                    start=False,
                    stop=(stop and kp == 3),
                    perf_mode=DR,
                )

        gram23(0, True)
        qsq_mm(1)
        gram23(1, True)

        # ------- sqrt(+||p||^2), negate, store (quarters pipelined) -----
        lt = sb.tile([CL, NQ], f32)
        for qi in range(4):
            s = slice(256 * qi, 256 * (qi + 1))
            nc.scalar.activation(
                lt[:, s], s_ps[:, s], AF.Sqrt, bias=pn_col[:, 0:1]
            )
            nc.vector.tensor_scalar_mul(lt[:, s], lt[:, s], -1.0)
            nc.sync.dma_start(out[:, s], lt[:, s])


def _build():
    if "nc" in _CACHE:
        return _CACHE["nc"]
    from concourse import bacc, mybir, tile

    f32 = mybir.dt.float32
    fp8 = mybir.dt.float8e4
    nc = bacc.Bacc(
        "TRN2",
        target_bir_lowering=False,
        debug=False,
        enable_asserts=False,
        num_devices=NCORES,
    )
    sup = nc.dram_tensor("sup", [128, 2 * SCH * 512], fp8, kind="ExternalInput").ap()
    qt = nc.dram_tensor("qt", [128, DCH * NQ], fp8, kind="ExternalInput").ap()
    out = nc.dram_tensor("out", [CL, NQ], f32, kind="ExternalOutput").ap()
    with tile.TileContext(nc) as tc:
        _emit(nc, tc, sup, qt, out)
    nc.compile()
    _CACHE["nc"] = nc
    return nc


def _shard(embeddings):
    import ml_dtypes

    emb = np.asarray(embeddings, dtype=np.float32).reshape(C, 2 * S, D)
    # support halves: classes 32b..32b+32, SLOT-major shard rows
    # (r = s*32 + c_local), swizzled [128, 2, SCH, 512] d-half-major
    # (row p of chunk j = shard row j*128+p), fp8
    sups = []
    for b in range(CB):
        shard = (
            emb[CL * b : CL * (b + 1), :S, :]
            .transpose(1, 0, 2)
            .reshape(CL * S, D)
        )
        sw = shard.reshape(SCH, 128, 2, 512).transpose(1, 2, 0, 3)
        sups.append(
            np.ascontiguousarray(
                sw.astype(ml_dtypes.float8_e4m3).reshape(128, 2 * SCH * 512)
            )
        )
    # query groups: rows 1024a..1024(a+1) of the query set, feature-major
    qry = emb[:, S:, :].reshape(C * S, D)
    qts = []
    for a in range(QA):
        q = qry[NQ * a : NQ * (a + 1)]
        qt_i = q.T.reshape(DCH, 128, NQ).transpose(1, 0, 2)
        qts.append(
            np.ascontiguousarray(
                qt_i.astype(ml_dtypes.float8_e4m3).reshape(128, DCH * NQ)
            )
        )
    in_maps = []
    for i in range(NCORES):
        a, b = i // CB, i % CB
        in_maps.append({"sup": sups[b], "qt": qts[a]})
    return in_maps


def _assemble(outs):
    """outs: per-core [CL, NQ] blocks -> full [C*S, C] logits."""
    logits = np.empty((C * S, C), dtype=np.float32)
    for i in range(NCORES):
        a, b = i // CB, i % CB
        logits[NQ * a : NQ * (a + 1), CL * b : CL * (b + 1)] = outs[i].T
    return logits


def kernel(embeddings_stacked, n_classes, n_support, **_unused):
    assert int(n_classes) == C and int(n_support) == S
    emb = np.asarray(embeddings_stacked)
    assert emb.shape == (C * 2 * S, D), emb.shape

    from concourse import bass_utils

    nc = _build()
    in_maps = _shard(emb)
    try:
        res = bass_utils.run_bass_kernel_spmd(
            nc, in_maps, core_ids=list(range(NCORES))
        )
    except Exception:
        # transient device/runtime hiccups have been observed; retry once
        res = bass_utils.run_bass_kernel_spmd(
            nc, in_maps, core_ids=list(range(NCORES))
        )
    return _assemble([res.results[i]["out"] for i in range(NCORES)])


if __name__ == "__main__":
    rng = np.random.default_rng(0)
    emb = rng.standard_normal((C * 2 * S, D), dtype=np.float32)
    got = kernel(emb, C, S)
    print("kernel output", got.shape, got.dtype)
